# revision 22
# baseline (speedup 1.0000x reference)
"""JointMLPDecoder TRN2 kernel: per-joint LayerNorm + MLP (D=512 -> 2048 -> 3).

Sharding: 24 joints split 3-per-core across 8 NeuronCores (expert-style).
Host packs x as x^T [J, D, B] in bf16 so each core streams [d, b] tiles.

Design (v2, single-pass bf16 GEMM1 — replaces the fp8 DoubleRow 3-term
scheme of v1):

The NTFF trace of v1 showed fp8 DoubleRow matmuls cost ~379 ns median at
N=512 (only ~1.44x bf16 MAC rate), so the 3-term hi/lo split (6 DR streams
per 128-col weight tile) costs ~2.1x what a single bf16 pass does (4 MMs of
K=128 @ ~216 ns). bf16 rounds x and w to ~8 mantissa bits; with K=512
incoherent accumulation the output error stays ~0.3% rms, far inside the
2e-2 gate. So:

1. GEMM1 in bf16: per (mc, chunk) 4 matmuls K=128, N=512. Host pre-centers
   weights (LayerNorm mean-subtraction + ln_g fold), so the device only
   applies rstd. FWL (automatic for bf16 128-col weights) keeps ldweights
   hidden behind the streams.

2. Per-joint batched stats: raw sums via bf16 ones-matmuls on x and x*x
   (x arrives bf16; x^2 on DVE), var evacuated per chunk into a per-joint
   [128, 8, 512] f32 tile, then ONE ACT Rsqrt(var + eps) per joint
   (reciprocal_sqrt table set; square/copy are in every set so only the
   rsqrt<->gelu swaps remain, ~2 per joint).

3. Chunk blocks of 4 share stationary weight tiles; the four per-chunk
   GEMM2 accumulators [3, 512] pack into ONE PSUM bank at partition
   offsets 0/32/64/96 (PE tile_position). 6 ring banks + 1 ps_y bank.

4. Stats of joint j+1 are emitted interleaved ahead of the two gemm blocks
   of joint j, so PE never waits on the DVE/ACT stats chain except at the
   prologue.

Per-block pipeline (4 batch chunks c of joint j):
  xs:     xs = x * rstd  [DVE, bf16]
  gemm1:  for mc in 16: for dc in 4: matmul(ph[c], w1[dc, mc], xs[c][dc])
  gelu:   h[c] = Gelu(ph[c] + b1[mc])  [ACT] -> f32r
  gemm2:  ps_y[3@32c, 512] += w2[mc].T @ h[c]  (f32r, lagged 1 mc)
  out:    y^T + b2 -> DRAM [3, 3, B] per core; host transposes to [B,1,24,3]
"""

import numpy as np
import ml_dtypes
from contextlib import ExitStack

import concourse.bass as bass
import concourse.bacc as bacc
import concourse.tile as tile
from concourse import mybir
from concourse import bass_utils

F32 = mybir.dt.float32
F32R = mybir.dt.float32r
BF16 = mybir.dt.bfloat16
F8 = mybir.dt.float8e4
NP_BF16 = ml_dtypes.bfloat16
NP_F8 = ml_dtypes.float8_e4m3
PM = mybir.MatmulPerfMode
AF = mybir.ActivationFunctionType
ALU = mybir.AluOpType

B = 4096
J = 24
D = 512
M = 2048
NCORES = 8
JPC = J // NCORES          # 3 joints per core
BCH = 512                  # batch chunk (matmul N)
NBC = B // BCH             # 8
NDC = D // 128             # 4 contraction chunks for gemm1
NMC = M // 128             # 16 contraction chunks for gemm2
BLK = 4                    # batch chunks per gemm block (shared stationaries)
EPS = 1e-5

_CACHE: dict = {}


def build_body(nc, tc, ctx, jpc=JPC, nbc=NBC, gelu=True):
    xT = nc.dram_tensor("xT", [jpc, D, B], BF16, kind="ExternalInput").ap()
    wa = nc.dram_tensor("wa", [jpc, 128, NDC, M], BF16, kind="ExternalInput").ap()
    b1 = nc.dram_tensor("b1", [jpc, 128, NMC], F32, kind="ExternalInput").ap()
    w2 = nc.dram_tensor("w2", [jpc, 128, NMC, 3], F32R, kind="ExternalInput").ap()
    b2 = nc.dram_tensor("b2", [jpc, 3, 1], F32, kind="ExternalInput").ap()
    ones = nc.dram_tensor("ones", [128, 2, 128], F8, kind="ExternalInput").ap()
    yT = nc.dram_tensor("yT", [jpc, 3, B], F32, kind="ExternalOutput").ap()

    consts = ctx.enter_context(tc.tile_pool(name="consts", bufs=1))
    wpool = ctx.enter_context(tc.tile_pool(name="wpool", bufs=2))
    xpool = ctx.enter_context(tc.tile_pool(name="xpool", bufs=10))
    sqpool = ctx.enter_context(tc.tile_pool(name="sqpool", bufs=4))
    xspool = ctx.enter_context(tc.tile_pool(name="xspool", bufs=8))
    spool = ctx.enter_context(tc.tile_pool(name="spool", bufs=2))
    stpool = ctx.enter_context(tc.tile_pool(name="stpool", bufs=4))
    hpool = ctx.enter_context(tc.tile_pool(name="hpool", bufs=8))
    opool = ctx.enter_context(tc.tile_pool(name="opool", bufs=4))
    ps_ring = ctx.enter_context(tc.tile_pool(name="ps_ring", bufs=4, space="PSUM"))
    ps_ypool = ctx.enter_context(tc.tile_pool(name="ps_y", bufs=1, space="PSUM"))

    ones_t = consts.tile([128, 2, 128], F8)
    nc.sync.dma_start(out=ones_t, in_=ones)
    eps_t = consts.tile([128, 1], F32)
    nc.vector.memset(eps_t, EPS)

    jw = {}    # per-joint weight tiles
    jx = {}    # per-(joint, chunk) x tiles
    jrstd = {} # per-joint rstd tiles

    def emit_load_joint(j):
        wa_t = wpool.tile([128, NDC, M], BF16, name="wa_t", tag="wa_t")
        nc.sync.dma_start(out=wa_t, in_=wa[j])
        w2_t = wpool.tile([128, NMC, 3], F32R, name="w2_t", tag="w2_t")
        nc.sync.dma_start(out=w2_t, in_=w2[j])
        b1_t = wpool.tile([128, NMC], F32, name="b1_t", tag="b1_t")
        nc.sync.dma_start(out=b1_t, in_=b1[j])
        b2_t = wpool.tile([3, 1], F32, name="b2_t", tag="b2_t")
        nc.sync.dma_start(out=b2_t, in_=b2[j])
        jw[j] = (wa_t, w2_t, b1_t, b2_t)

    def emit_stats_block(j, bcs):
        """Stats for one 4-chunk block, then rstd = exp(-0.5*ln(var+eps));
        Ln and Exp share the natural_log_exp_and_others ACT table set
        (Rsqrt is blocked by bass).

        Two passes: first every chunk's x load + x^2 + raw-sum matmuls (so
        the PE stats matmuls never wait on evacuation work), then every
        chunk's mu/var evacuation."""
        n = len(bcs)
        var_b = spool.tile([128, n, BCH], F32, name="var_b", tag="var_b")
        work = []
        for c, bc in enumerate(bcs):
            bsl = slice(bc * BCH, (bc + 1) * BCH)
            xt = xpool.tile([128, NDC, BCH], BF16, name="xt", tag="xt")
            dma_eng = nc.sync if c % 2 == 0 else nc.gpsimd
            dma_eng.dma_start(
                out=xt,
                in_=xT[j, :, bsl].rearrange("(dc p) b -> p dc b", p=128),
            )
            jx[(j, bc)] = xt

            xq = sqpool.tile([128, NDC, BCH], F8, name="xq", tag="xq")
            nc.vector.tensor_copy(xq.rearrange("p n b -> p (n b)"),
                                  xt.rearrange("p n b -> p (n b)"))
            xsq = sqpool.tile([128, NDC, BCH], F8, name="xsq", tag="xsq")
            nc.vector.tensor_mul(xsq.rearrange("p n b -> p (n b)"),
                                 xt.rearrange("p n b -> p (n b)"),
                                 xt.rearrange("p n b -> p (n b)"))

            # raw sums over d: fp8 DoubleRow ones-matmuls, K=256 each
            ps_mu = ps_ring.tile([128, BCH], F32, name="ps_mu", tag="ps_h")
            ps_ms = ps_ring.tile([128, BCH], F32, name="ps_ms", tag="ps_h")
            for i in range(NDC // 2):
                nc.tensor.matmul(ps_mu, ones_t, xq[:, 2 * i:2 * i + 2, :],
                                 start=(i == 0), stop=(i == NDC // 2 - 1),
                                 perf_mode=PM.DoubleRow)
            for i in range(NDC // 2):
                nc.tensor.matmul(ps_ms, ones_t, xsq[:, 2 * i:2 * i + 2, :],
                                 start=(i == 0), stop=(i == NDC // 2 - 1),
                                 perf_mode=PM.DoubleRow)
            work.append((ps_mu, ps_ms, c))

        for ps_mu, ps_ms, c in work:
            # evac on DVE: ACT's queue (64 gelus/block) releases ring-bank
            # WARs far too late; DVE is idle during the stats window.
            mu_t = stpool.tile([128, BCH], F32, name="mu_t", tag="mu_t")
            nc.vector.tensor_copy(mu_t, ps_mu)
            msq_t = stpool.tile([128, BCH], F32, name="msq_t", tag="msq_t")
            nc.vector.scalar_tensor_tensor(
                out=msq_t, in0=mu_t, scalar=-1.0 / (512.0 * 512.0), in1=mu_t,
                op0=ALU.mult, op1=ALU.mult)
            nc.vector.scalar_tensor_tensor(
                out=var_b[:, c, :], in0=ps_ms, scalar=1.0 / 512.0, in1=msq_t,
                op0=ALU.mult, op1=ALU.add)

        rstd = spool.tile([128, n, BCH], BF16, name="rstd", tag="rstd")
        v_flat = var_b.rearrange("p n b -> p (n b)")
        nc.scalar.activation(v_flat, v_flat, AF.Ln, bias=eps_t, scale=1.0)
        nc.scalar.activation(
            rstd.rearrange("p n b -> p (n b)"), v_flat, AF.Exp, scale=-0.5)
        jrstd[(j, tuple(bcs))] = rstd

    jxs = {}

    def emit_xs_block(j, bcs):
        """xs = x * rstd for one block; emitted one full gemm block ahead so
        the DVE runs it while PE is busy and never stalls a block start."""
        rstd = jrstd.pop((j, tuple(bcs)))
        for c, bc in enumerate(bcs):
            xt = jx.pop((j, bc))
            xs = xspool.tile([128, NDC, BCH], BF16, name="xs", tag="xs")
            for dc in range(NDC):
                nc.vector.tensor_mul(xs[:, dc, :], xt[:, dc, :], rstd[:, c, :])
            jxs[(j, bc)] = xs

    def emit_gemm_block(j, bcs):
        """One gemm block: 4 chunks of joint j sharing stationary tiles."""
        wa_t, w2_t, b1_t, b2_t = jw[j]
        n = len(bcs)
        xss = [jxs.pop((j, bc)) for bc in bcs]

        ps_ys = [ps_ypool.tile([3, BCH], F32, name=f"ps_y{ci}", tag=f"ps_y{ci}")
                 for ci in range(n)]
        h_prev = []

        def emit_g2(mc, h_list):
            for ci, h_t in h_list:
                nc.tensor.matmul(ps_ys[ci], w2_t[:, mc, :], h_t,
                                 start=(mc == 0), stop=(mc == NMC - 1))

        for mc in range(NMC):
            msl = slice(mc * 128, (mc + 1) * 128)
            ph = [ps_ring.tile([128, BCH], F32, name=f"ps_h{ci}", tag="ps_h")
                  for ci in range(n)]
            for dc in range(NDC):
                w_ap = wa_t[:, dc, msl]
                for ci in range(n):
                    nc.tensor.matmul(ph[ci], w_ap, xss[ci][:, dc, :],
                                     start=(dc == 0), stop=(dc == NDC - 1))
            h_list = []
            for ci in range(n):
                h_t = hpool.tile([128, BCH], F32R, name="h_t", tag="h_t")
                nc.scalar.activation(h_t, ph[ci],
                                     AF.Gelu if gelu else AF.Identity,
                                     bias=b1_t[:, mc:mc + 1], scale=1.0)
                h_list.append((ci, h_t))
            if mc >= 1:
                emit_g2(mc - 1, h_prev)
            h_prev = h_list
        emit_g2(NMC - 1, h_prev)

        for ci, bc in enumerate(bcs):
            bsl = slice(bc * BCH, (bc + 1) * BCH)
            y_sb = opool.tile([3, BCH], F32, name="y_sb", tag="y_sb")
            # +b2 on ACT (Identity is in every table set) so the DVE queue
            # never stalls behind the block's last gemm2 matmul.
            nc.scalar.activation(y_sb, ps_ys[ci], AF.Identity,
                                 bias=b2_t, scale=1.0)
            nc.gpsimd.dma_start(out=yT[j, :, bsl], in_=y_sb)

    # ---- software pipeline over 4-chunk blocks -----------------------
    # block X runs stats(X+1) + xs(X+1) interleaved ahead of gemm(X) so
    # every feeder has a full gemm block (~80us) of lead.
    blocks = []
    for j in range(jpc):
        for b0 in range(0, nbc, BLK):
            blocks.append((j, list(range(b0, min(b0 + BLK, nbc)))))
    emit_stats_block(*blocks[0])
    emit_load_joint(0)
    emit_xs_block(*blocks[0])
    for X, (j, bcs) in enumerate(blocks):
        if bcs[0] == 0 and j + 1 < jpc:
            emit_load_joint(j + 1)
        if X + 1 < len(blocks):
            emit_stats_block(*blocks[X + 1])
            emit_xs_block(*blocks[X + 1])
        emit_gemm_block(j, bcs)


def _build_nc(jpc=JPC, nbc=NBC, reps=1, gelu=True):
    nc = bacc.Bacc("TRN2", target_bir_lowering=False, debug=False, num_devices=NCORES)
    with tile.TileContext(nc) as tc, ExitStack() as ctx:
        if reps == 1:
            build_body(nc, tc, ctx, jpc, nbc, gelu)
        else:
            # timing variant: repeat the whole body in a hardware loop
            def body(_i, unroll=1):
                with ExitStack() as c2:
                    build_body(nc, tc, c2, jpc, nbc, gelu)
            with tc.For_i(0, reps, 1, staggered_reset=True) as i:
                body(i)
    nc.compile()
    return nc


def _pack_inputs(x, ln_g, ln_b, w1, b1, w2, b2):
    x = np.asarray(x, dtype=np.float32)
    w1 = np.asarray(w1, dtype=np.float32)
    b1 = np.asarray(b1, dtype=np.float32)
    w2 = np.asarray(w2, dtype=np.float32)
    b2 = np.asarray(b2, dtype=np.float32)
    ln_g = np.asarray(ln_g, dtype=np.float32)
    ln_b = np.asarray(ln_b, dtype=np.float32)

    # fold LN affine + mean-subtraction into centered weights
    w1g = ln_g[:, :, None] * w1
    w1c = w1g - w1g.sum(axis=1, keepdims=True) / D                # [J, D, M]
    b1e = b1 + np.einsum("jd,jdm->jm", ln_b, w1g)

    # lhsT layout [J, 128, NDC, M]: wa[j, p, dc, m] = w1c[j, dc*128+p, m]
    wa = np.ascontiguousarray(
        w1c.reshape(J, NDC, 128, M).transpose(0, 2, 1, 3)).astype(NP_BF16)

    xT = np.ascontiguousarray(x.transpose(1, 2, 0)).astype(NP_BF16)  # [J, D, B]
    w2p = np.ascontiguousarray(
        w2.reshape(J, NMC, 128, 3).transpose(0, 2, 1, 3))    # [J, 128, NMC, 3]
    b1p = np.ascontiguousarray(
        b1e.reshape(J, NMC, 128).transpose(0, 2, 1))         # [J, 128, NMC]
    b2p = np.ascontiguousarray(b2.reshape(J, 3, 1))
    ones = np.full((128, 2, 128), 1.0, dtype=NP_F8)

    in_maps = []
    for c in range(NCORES):
        js = slice(c * JPC, (c + 1) * JPC)
        in_maps.append({
            "xT": np.ascontiguousarray(xT[js]),
            "wa": np.ascontiguousarray(wa[js]),
            "b1": b1p[js],
            "w2": w2p[js],
            "b2": b2p[js],
            "ones": ones,
        })
    return in_maps


def kernel(x, ln_g, ln_b, w1, b1, w2, b2):
    if "nc" not in _CACHE:
        _CACHE["nc"] = _build_nc()
    nc = _CACHE["nc"]

    in_maps = _pack_inputs(x, ln_g, ln_b, w1, b1, w2, b2)
    res = bass_utils.run_bass_kernel_spmd(nc, in_maps, core_ids=list(range(NCORES)))

    # yT per core: [JPC, 3, B] -> y [B, 1, J, 3]
    yT = np.stack([res.results[c]["yT"] for c in range(NCORES)])  # [8, JPC, 3, B]
    y = yT.reshape(J, 3, B).transpose(2, 0, 1)[:, None, :, :]
    return np.ascontiguousarray(y.astype(np.float32))
